# revision 1
# baseline (speedup 1.0000x reference)
"""Trainium2 Bass kernel for a 2-layer LSTM (64, 32) + MLP head.

Model (PyTorch semantics, eval mode):
    h1 = LSTM(4 -> 64)(x)            x: [B=4096, T=512, 4]
    h2 = LSTM(64 -> 32)(h1)
    y  = (relu(h2[:, -1] @ w_fc1.T + b_fc1)) @ w_fc2.T + b_fc2   # [B, 1]

Sharding: data-parallel over batch across 8 NeuronCores (512 rows each),
weights replicated. Inside each core the state is kept *transposed*
([units, batch]) so the per-timestep recurrent matmuls have batch on the
moving free dimension (N=512) and the gate nonlinearities run as a few
wide ops on full 96-partition stacks (layer-1 and layer-2 gates stacked).

State tile S [97, 512]: rows 0:64 = h1^T, rows 64:96 = h2^T, row 96 =
ones (bias row).  Both layers' recurrent matmuls use rhs S[0:97] (base
partition 0 — the PE moving operand must start at 0 to span >32
partitions); layer-1's weight rows over the h2 region are zeros, and
layer-2's over nothing (it genuinely uses h1+h2).  Biases ride the
ones-row through the matmul (incl. the fc1 bias in the head).
The input projection is a K=4 matmul per gate accumulating into the
same PSUM bank; x_t arrives per step by DMA into a small [4, 512]
rotating tile (x is recurrence-independent, so these prefetch ahead and
the matmuls run one step early, filling the TensorE pipe during the
previous step's ACT/DVE chain).

Each gate gets its own PSUM tile (per-tile dependency tracking lets each
sigmoid start as soon as its own gate's matmuls finish); layer-1 and
layer-2 are fused into one M=96 matmul per gate (both contract the same
rhs S[0:97]).  Gate order f,i,g,o: sigmoid(f) (and f*c) overlap the
remaining matmuls; sigmoid(o) fills the ACT gap while the vector engine
runs the cell update; tanh(c) and h close the serial chain.
"""

import numpy as np
from contextlib import ExitStack

import concourse.bass as bass
import concourse.tile as tile
from concourse import bacc, mybir
from concourse import bass_utils

AF = mybir.ActivationFunctionType

B, T, D_IN, H1, H2 = 4096, 512, 4, 64, 32
NCORES = 8
BL = B // NCORES  # 512 batch rows per core

F32 = mybir.dt.float32
# Compute dtypes (flip for perf/accuracy trades):
DT = mybir.dt.bfloat16  # weights / state / gate-activation dtype
CDT = mybir.dt.bfloat16  # cell-state dtype

HS = H1 + H2  # 96: stacked (layer1, layer2) partition extent


def _build(n_steps: int = T):
    """Build the SPMD single-core Bass program (same NEFF on all 8 cores)."""
    nc = bacc.Bacc("TRN2", target_bir_lowering=False, debug=False)

    xT = nc.dram_tensor("xT", [n_steps * 4, BL], DT, kind="ExternalInput")
    w12t = nc.dram_tensor("w12t", [97, 4 * HS], DT, kind="ExternalInput")
    w1x = nc.dram_tensor("w1x", [4, 4 * HS], DT, kind="ExternalInput")
    wf1 = nc.dram_tensor("wf1", [97, 16], DT, kind="ExternalInput")
    wf2 = nc.dram_tensor("wf2", [16, 1], DT, kind="ExternalInput")
    bf2 = nc.dram_tensor("bf2", [1, 1], F32, kind="ExternalInput")
    out = nc.dram_tensor("out", [1, BL], F32, kind="ExternalOutput")

    with tile.TileContext(nc) as tc, ExitStack() as ctx:
        const = ctx.enter_context(tc.tile_pool(name="const", bufs=1))
        xpool = ctx.enter_context(tc.tile_pool(name="xp", bufs=8))
        gates = ctx.enter_context(tc.tile_pool(name="gates", bufs=3))

        W12 = const.tile([97, 4 * HS], DT, tag="W12")
        nc.sync.dma_start(W12[:], w12t.ap())
        W1X = const.tile([4, 4 * HS], DT, tag="W1X")
        nc.sync.dma_start(W1X[:], w1x.ap())
        WF1 = const.tile([97, 16], DT, tag="WF1")
        nc.sync.dma_start(WF1[:], wf1.ap())
        WF2 = const.tile([16, 1], DT, tag="WF2")
        nc.sync.dma_start(WF2[:], wf2.ap())
        BF2 = const.tile([1, 1], F32, tag="BF2")
        nc.sync.dma_start(BF2[:], bf2.ap())

        S = const.tile([97, BL], DT, tag="S")
        C = const.tile([HS, BL], CDT, tag="C")
        nc.vector.memset(S[:], 0.0)
        nc.vector.memset(S[96:97, :], 1.0)
        nc.vector.memset(C[:], 0.0)

        # Per-gate PSUM tiles (per-bank dependency tracking, so each
        # activation op starts as soon as its own gate's matmuls finish):
        # Pf [96,512] (f), Pio [96,1024] (i|o), Pg [96,512] (g).
        # Layer-1 (cols 0:64 of each gate's weight block) and layer-2
        # (cols 64:96) are fused into ONE M=96 matmul per gate — they
        # share the rhs S[0:97].  The x-projection is a K=4 matmul per
        # gate (M=96, layer-2 columns zero) emitted one step AHEAD
        # (start=True), so it fills the TensorE pipe during the previous
        # step's ACT/DVE chain; the recurrent matmul accumulates on top.
        #
        # Software-pipelined over layers: at iteration k the layer-1
        # partition computes LSTM-1 step k while the layer-2 partition
        # computes LSTM-2 step k-1 (both read h1_{k-1} from S).
        # Iteration 0 produces garbage layer-2 state (cleared after);
        # iteration n_steps produces garbage layer-1 state (the head
        # weights are zero over the h1 rows).
        GSEL = {"i": 0, "f": 1, "g": 2, "o": 3}

        def alloc_P():
            Pf = psum.tile([HS, BL], F32, tag="Pf")
            Pi = psum.tile([HS, BL], F32, tag="Pi")
            Pg = psum.tile([HS, BL], F32, tag="Pg")
            Po = psum.tile([HS, BL], F32, tag="Po")
            # (gate, dest-ap) in emission order: f, i, g, o —
            # f first (feeds f*c as early as possible), o last (only
            # needed at the very end for h = o * tanh(c)).
            return [
                ("f", Pf[:, :]),
                ("i", Pi[:, :]),
                ("g", Pg[:, :]),
                ("o", Po[:, :]),
            ], Pf, Pi, Pg, Po

        def emit_x_mms(banks, step):
            XTT = xpool.tile([4, BL], DT, tag="xt")
            nc.sync.dma_start(XTT[:], xT.ap()[4 * step : 4 * step + 4, :])
            for gate, dest in banks:
                gsel = GSEL[gate]
                nc.tensor.matmul(
                    dest,
                    W1X[:, gsel * HS : (gsel + 1) * HS],
                    XTT[:],
                    start=True,
                    stop=False,
                )

        with tc.tile_pool(name="psum", bufs=2, space="PSUM") as psum:
            banks, Pf, Pi, Pg, Po = alloc_P()
            emit_x_mms(banks, 0)
            for k in range(n_steps + 1):
                has_x = k < n_steps  # P already holds the x contribution
                for gate, dest in banks:
                    gsel = GSEL[gate]
                    nc.tensor.matmul(
                        dest,
                        W12[:, gsel * HS : (gsel + 1) * HS],
                        S[0:97, :],
                        start=not has_x,
                        stop=True,
                    )

                if k + 1 <= n_steps:
                    nbanks, nPf, nPi, nPg, nPo = alloc_P()
                    if k + 1 < n_steps:
                        emit_x_mms(nbanks, k + 1)

                SIGF = gates.tile([HS, BL], DT, tag="SIGF")
                SIGI = gates.tile([HS, BL], DT, tag="SIGI")
                G = gates.tile([HS, BL], DT, tag="G")
                SIGO = gates.tile([HS, BL], DT, tag="SIGO")
                nc.scalar.activation(SIGF[:], Pf[:, :], AF.Sigmoid)
                nc.scalar.activation(SIGI[:], Pi[:, :], AF.Sigmoid)
                nc.scalar.activation(G[:], Pg[:, :], AF.Tanh)
                nc.scalar.activation(SIGO[:], Po[:, :], AF.Sigmoid)

                U = gates.tile([HS, BL], DT, tag="U")
                V = gates.tile([HS, BL], CDT, tag="V")
                nc.vector.tensor_mul(V[:], SIGF[:], C[:])               # f*c
                nc.vector.tensor_mul(U[:], SIGI[:], G[:])               # i*g
                nc.vector.tensor_add(C[:], U[:], V[:])                  # c'
                TC = gates.tile([HS, BL], DT, tag="TC")
                nc.scalar.activation(TC[:], C[:], AF.Tanh)
                nc.vector.tensor_mul(S[0:HS, :], SIGO[:], TC[:])        # h
                if k == 0:
                    # wipe the garbage layer-2 state from the pipeline warmup
                    nc.vector.memset(S[H1:HS, :], 0.0)
                    nc.vector.memset(C[H1:HS, :], 0.0)
                if k + 1 <= n_steps:
                    banks, Pf, Pi, Pg, Po = nbanks, nPf, nPi, nPg, nPo

        # MLP head on h2 at the last timestep (rows 64:96 of S).
        with tc.tile_pool(name="psum_head", bufs=1, space="PSUM") as psh:
            PF = psh.tile([16, BL], F32, tag="PF")
            nc.tensor.matmul(PF[:], WF1[:, :], S[0:97, :], start=True, stop=True)
            Z = gates.tile([16, BL], DT, tag="Z")
            nc.scalar.activation(Z[:], PF[:], AF.Relu)
            PO = psh.tile([1, BL], F32, tag="PO")
            nc.tensor.matmul(PO[:], WF2[:, :], Z[:], start=True, stop=True)
            Y = gates.tile([1, BL], F32, tag="Y")
            nc.scalar.activation(Y[:], PO[:], AF.Identity, bias=BF2[:, 0:1])
            nc.sync.dma_start(out.ap(), Y[:])

    nc.compile()
    return nc


def _pack_weights(inputs, np_dt):
    w_ih1, w_hh1 = inputs["w_ih1"], inputs["w_hh1"]
    w_ih2, w_hh2 = inputs["w_ih2"], inputs["w_hh2"]
    b1 = (inputs["b_ih1"] + inputs["b_hh1"]).astype(np.float32)
    b2 = (inputs["b_ih2"] + inputs["b_hh2"]).astype(np.float32)
    # Layer-1 gate weights as [97, 256]: rows = [w_hh1^T(64); zeros(32);
    # bias1(1)] matching rhs S[0:97] = [h1; h2(ignored); ones].
    z32 = np.zeros((4 * H1, 32), np.float32)
    w1t = np.concatenate([w_hh1, z32, b1[:, None]], axis=1).T
    # Layer-2 gate weights as [97, 128]: rows = [w_ih2^T(64); w_hh2^T(32);
    # bias2(1)].
    w2t = np.concatenate([w_ih2, w_hh2, b2[:, None]], axis=1).T
    # Fused per-gate blocks [97, 96]: layer-1 output units in cols 0:64,
    # layer-2 in cols 64:96 (one M=96 matmul per gate).
    w12t = np.concatenate(
        [np.concatenate([w1t[:, g * H1 : (g + 1) * H1],
                         w2t[:, g * H2 : (g + 1) * H2]], axis=1)
         for g in range(4)], axis=1)
    # Input projection [4, 384]: per gate [w_ih1^T (64) | zeros (32)].
    zx = np.zeros((4, H2), np.float32)
    w1x = np.concatenate(
        [np.concatenate([w_ih1.T[:, g * H1 : (g + 1) * H1], zx], axis=1)
         for g in range(4)], axis=1)
    return {
        "w12t": np.ascontiguousarray(w12t).astype(np_dt),
        "w1x": np.ascontiguousarray(w1x).astype(np_dt),
        "wf1": np.ascontiguousarray(np.concatenate(
            [np.zeros((64, 16), np.float32), inputs["w_fc1"].T,
             inputs["b_fc1"][None, :]], axis=0)).astype(np_dt),
        "wf2": np.ascontiguousarray(inputs["w_fc2"].T).astype(np_dt),
        "bf2": np.ascontiguousarray(inputs["b_fc2"][:, None]).astype(np.float32),
    }


_built = {}


def _get_nc(n_steps):
    if n_steps not in _built:
        _built[n_steps] = _build(n_steps)
    return _built[n_steps]


def _run(inputs, n_steps=T, **run_kwargs):
    np_dt = mybir.dt.np(DT)
    x = np.asarray(inputs["x"], np.float32)
    nb = x.shape[0]
    ncores = NCORES
    bl = nb // ncores
    assert bl == BL and x.shape[1] >= n_steps
    shared = _pack_weights({k: np.asarray(v, np.float32) for k, v in inputs.items()
                            if k != "x"} | {}, np_dt)
    in_maps = []
    for c in range(ncores):
        xs = x[c * bl : (c + 1) * bl, :n_steps, :]  # [BL, T, 4]
        xT = np.ascontiguousarray(xs.transpose(1, 2, 0).reshape(n_steps * 4, bl))
        in_maps.append(dict(shared, xT=xT.astype(np_dt)))
    nc = _get_nc(n_steps)
    res = bass_utils.run_bass_kernel_spmd(
        nc, in_maps, core_ids=list(range(ncores)), **run_kwargs
    )
    y = np.concatenate(
        [np.asarray(r["out"], np.float32).reshape(bl, 1) for r in res.results], axis=0
    )
    return y, res


def kernel(**inputs) -> np.ndarray:
    y, _ = _run(inputs)
    return y



# revision 3
# speedup vs baseline: 1.1074x; 1.1074x over previous
"""Trainium2 Bass kernel for a 2-layer LSTM (64, 32) + MLP head.

Model (PyTorch semantics, eval mode):
    h1 = LSTM(4 -> 64)(x)            x: [B=4096, T=512, 4]
    h2 = LSTM(64 -> 32)(h1)
    y  = (relu(h2[:, -1] @ w_fc1.T + b_fc1)) @ w_fc2.T + b_fc2   # [B, 1]

Sharding: data-parallel over batch across 8 NeuronCores (512 rows each),
weights replicated.

v2 design (vs the v1 per-gate/PSUM-bank pipeline):
  * State kept transposed [units, batch]; layer-1 (64) and layer-2 (32)
    unit stacks fused to 96 rows with the 1-step layer-2 lag.
  * x folded INTO the recurrent matmul: the moving operand S is
    [101, F]: rows 0:96 h, row 96 ones (bias), rows 97:101 x_t.  One
    matmul per gate per step (K=101) instead of recurrent + x-proj
    pairs -- halves TensorE moving columns.
  * All 4 gates' matmuls write one contiguous PSUM tile [96, 4F], so
    ONE sigmoid activation instruction covers all four gates (ACT's
    ~185ns per-instruction access latency amortized 4x).  tanh(g) is
    computed as 2*sigmoid(2g)-1: the x2 is folded into g's weights, and
    the affine fix-up is a 4x-rate DVE tensor_scalar op.
  * Batch split into 2 independent chains of 256 (separate S/C/PSUM),
    interleaved in emission order so each engine works on one chain
    while the other chain's serial chain (mm -> sigmoid -> cell -> tanh
    -> h -> mm) is in flight on other engines.
  * S rotates over 4 buffers per chain; x_t DMA lands 4 steps ahead.
"""

import numpy as np
from contextlib import ExitStack

import concourse.bass as bass
import concourse.tile as tile
from concourse import bacc, mybir
from concourse import bass_utils
from concourse.alu_op_type import AluOpType

AF = mybir.ActivationFunctionType

B, T, D_IN, H1, H2 = 4096, 512, 4, 64, 32
NCORES = 8
BL = B // NCORES  # 512 batch rows per core

F32 = mybir.dt.float32
DT = mybir.dt.bfloat16

HS = H1 + H2          # 96 stacked units
KR = HS + 1 + D_IN    # 101 = h rows + ones row + x rows
NCH = 2               # batch chains per core
FC = BL // NCH        # 256 free columns per chain
NSB = 4               # S rotation depth


def _build(n_steps: int = T):
    nc = bacc.Bacc("TRN2", target_bir_lowering=False, debug=False)

    xT = nc.dram_tensor("xT", [n_steps * 4, BL], DT, kind="ExternalInput")
    wg = nc.dram_tensor("wg", [KR, 4 * HS], DT, kind="ExternalInput")
    wf1 = nc.dram_tensor("wf1", [HS + 1, 16], DT, kind="ExternalInput")
    wf2 = nc.dram_tensor("wf2", [16, 1], DT, kind="ExternalInput")
    bf2 = nc.dram_tensor("bf2", [1, 1], F32, kind="ExternalInput")
    out = nc.dram_tensor("out", [1, BL], F32, kind="ExternalOutput")

    with tile.TileContext(nc) as tc, ExitStack() as ctx:
        const = ctx.enter_context(tc.tile_pool(name="const", bufs=1))
        sg_pool = ctx.enter_context(tc.tile_pool(name="sg", bufs=2))
        cell = ctx.enter_context(tc.tile_pool(name="cell", bufs=2))

        W = const.tile([KR, 4 * HS], DT, tag="W")
        nc.sync.dma_start(W[:], wg.ap())
        WF1 = const.tile([HS + 1, 16], DT, tag="WF1")
        nc.sync.dma_start(WF1[:], wf1.ap())
        WF2 = const.tile([16, 1], DT, tag="WF2")
        nc.sync.dma_start(WF2[:], wf2.ap())
        BF2 = const.tile([1, 1], F32, tag="BF2")
        nc.sync.dma_start(BF2[:], bf2.ap())

        # Per-chain state: S rotation ring and cell state C.
        S = [[None] * NSB for _ in range(NCH)]
        C = [None] * NCH
        for ch in range(NCH):
            for j in range(NSB):
                Sj = const.tile([KR, FC], DT, tag=f"S{ch}_{j}")
                nc.vector.memset(Sj[:], 0.0)
                nc.vector.memset(Sj[HS : HS + 1, :], 1.0)
                S[ch][j] = Sj
            Cch = const.tile([HS, FC], DT, tag=f"C{ch}")
            nc.vector.memset(Cch[:], 0.0)
            C[ch] = Cch

        # Preload x for steps 0..NSB-1 into the rings.
        for j in range(min(NSB, n_steps)):
            for ch in range(NCH):
                nc.sync.dma_start(
                    S[ch][j][HS + 1 : KR, :],
                    xT.ap()[4 * j : 4 * j + 4, ch * FC : (ch + 1) * FC],
                )

        # Gate order in W columns: i, f, g, o (each HS=96 wide).
        GI, GF, GG, GO = 0, 1, 2, 3

        with tc.tile_pool(name="psum", bufs=1, space="PSUM") as psum:
            P = [psum.tile([HS, 4 * FC], F32, tag=f"P{ch}", name=f"P{ch}") for ch in range(NCH)]

            for k in range(n_steps + 1):
                cur = [S[ch][k % NSB] for ch in range(NCH)]
                nxt = [S[ch][(k + 1) % NSB] for ch in range(NCH)]

                # 1) Recurrent+input matmuls: all 4 gates into one PSUM tile.
                for ch in range(NCH):
                    for g in range(4):
                        nc.tensor.matmul(
                            P[ch][:, g * FC : (g + 1) * FC],
                            W[:, g * HS : (g + 1) * HS],
                            cur[ch][0:KR, :],
                            start=True,
                            stop=True,
                        )
                # Prefetch x for step k+NSB into the buffer just freed.
                if k + NSB < n_steps:
                    for ch in range(NCH):
                        nc.sync.dma_start(
                            cur[ch][HS + 1 : KR, :],
                            xT.ap()[
                                4 * (k + NSB) : 4 * (k + NSB) + 4,
                                ch * FC : (ch + 1) * FC,
                            ],
                        )

                # 2) One sigmoid over all four gates per chain.
                SG = [None] * NCH
                for ch in range(NCH):
                    SGc = sg_pool.tile([HS, 4 * FC], DT, tag=f"SG{ch}")
                    nc.scalar.activation(SGc[:], P[ch][:, :], AF.Sigmoid)
                    SG[ch] = SGc

                # 3) Cell update on DVE (per chain, interleaved emission).
                GN = [None] * NCH
                UU = [None] * NCH
                VV = [None] * NCH
                for ch in range(NCH):
                    GNc = cell.tile([HS, FC], DT, tag=f"GN{ch}")
                    # g = tanh(g_pre) = 2*sigmoid(2*g_pre) - 1 (x2 in weights)
                    nc.vector.tensor_scalar(
                        GNc[:], SG[ch][:, GG * FC : (GG + 1) * FC],
                        2.0, 1.0, AluOpType.mult, AluOpType.subtract,
                    )
                    GN[ch] = GNc
                for ch in range(NCH):
                    Uc = cell.tile([HS, FC], DT, tag=f"U{ch}")
                    nc.vector.tensor_mul(
                        Uc[:], SG[ch][:, GI * FC : (GI + 1) * FC], GN[ch][:]
                    )
                    UU[ch] = Uc
                    Vc = cell.tile([HS, FC], DT, tag=f"V{ch}")
                    nc.vector.tensor_mul(
                        Vc[:], SG[ch][:, GF * FC : (GF + 1) * FC], C[ch][:]
                    )
                    VV[ch] = Vc
                    nc.vector.tensor_add(C[ch][:], UU[ch][:], VV[ch][:])

                # 4) tanh(c) on ACT, then h on DVE into the next S buffer.
                TC = [None] * NCH
                for ch in range(NCH):
                    TCc = cell.tile([HS, FC], DT, tag=f"TC{ch}")
                    nc.scalar.activation(TCc[:], C[ch][:], AF.Tanh)
                    TC[ch] = TCc
                for ch in range(NCH):
                    nc.vector.tensor_mul(
                        nxt[ch][0:HS, :], SG[ch][:, GO * FC : (GO + 1) * FC],
                        TC[ch][:],
                    )
                if k == 0:
                    # Wipe garbage layer-2 state from the pipeline warmup.
                    for ch in range(NCH):
                        nc.vector.memset(nxt[ch][H1:HS, :], 0.0)
                        nc.vector.memset(C[ch][H1:HS, :], 0.0)

        # MLP head on h2 of the final state (rows 64:96; wf1 zero over h1).
        final = [S[ch][(n_steps + 1) % NSB] for ch in range(NCH)]
        with tc.tile_pool(name="psum_head", bufs=1, space="PSUM") as psh:
            for ch in range(NCH):
                PF = psh.tile([16, FC], F32, tag=f"PF{ch}")
                nc.tensor.matmul(
                    PF[:], WF1[:, :], final[ch][0 : HS + 1, :],
                    start=True, stop=True,
                )
                Z = cell.tile([16, FC], DT, tag=f"Z{ch}")
                nc.scalar.activation(Z[:], PF[:], AF.Relu)
                PO = psh.tile([1, FC], F32, tag=f"PO{ch}")
                nc.tensor.matmul(PO[:], WF2[:, :], Z[:], start=True, stop=True)
                Y = cell.tile([1, FC], F32, tag=f"Y{ch}")
                nc.scalar.activation(Y[:], PO[:], AF.Identity, bias=BF2[:, 0:1])
                nc.sync.dma_start(out.ap()[:, ch * FC : (ch + 1) * FC], Y[:])

    nc.compile()
    return nc


def _pack_weights(inputs, np_dt):
    w_ih1, w_hh1 = inputs["w_ih1"], inputs["w_hh1"]
    w_ih2, w_hh2 = inputs["w_ih2"], inputs["w_hh2"]
    b1 = (inputs["b_ih1"] + inputs["b_hh1"]).astype(np.float32)
    b2 = (inputs["b_ih2"] + inputs["b_hh2"]).astype(np.float32)
    # Fused gate weight blocks [KR=101, 4*HS].  Rows: h1(64), h2(32),
    # ones/bias(1), x(4).  Columns gate-major i,f,g,o; within a gate,
    # layer-1 units then layer-2 units.  Layer 1 uses h1+x rows; layer 2
    # uses h1 (=w_ih2) + h2 rows and no x.  g gate scaled by 2 so that
    # tanh(g) = 2*sigmoid(2g) - 1 needs only sigmoid.
    blocks = []
    for g in range(4):
        w1 = np.concatenate(
            [
                w_hh1[g * H1 : (g + 1) * H1, :].T,           # h1 rows [64,64]
                np.zeros((H2, H1), np.float32),              # h2 rows
                b1[None, g * H1 : (g + 1) * H1],             # bias row
                w_ih1[g * H1 : (g + 1) * H1, :].T,           # x rows [4,64]
            ],
            axis=0,
        )
        w2 = np.concatenate(
            [
                w_ih2[g * H2 : (g + 1) * H2, :].T,           # h1 rows [64,32]
                w_hh2[g * H2 : (g + 1) * H2, :].T,           # h2 rows [32,32]
                b2[None, g * H2 : (g + 1) * H2],             # bias row
                np.zeros((D_IN, H2), np.float32),            # x rows
            ],
            axis=0,
        )
        blk = np.concatenate([w1, w2], axis=1)               # [101, 96]
        if g == 2:
            blk = blk * 2.0
        blocks.append(blk)
    wg = np.concatenate(blocks, axis=1)                      # [101, 384]
    return {
        "wg": np.ascontiguousarray(wg).astype(np_dt),
        "wf1": np.ascontiguousarray(
            np.concatenate(
                [np.zeros((H1, 16), np.float32), inputs["w_fc1"].T,
                 inputs["b_fc1"][None, :]], axis=0)
        ).astype(np_dt),
        "wf2": np.ascontiguousarray(inputs["w_fc2"].T).astype(np_dt),
        "bf2": np.ascontiguousarray(inputs["b_fc2"][:, None]).astype(np.float32),
    }


_built = {}


def _get_nc(n_steps):
    if n_steps not in _built:
        _built[n_steps] = _build(n_steps)
    return _built[n_steps]


def _run(inputs, n_steps=T, **run_kwargs):
    np_dt = mybir.dt.np(DT)
    x = np.asarray(inputs["x"], np.float32)
    nb = x.shape[0]
    ncores = NCORES
    bl = nb // ncores
    assert bl == BL and x.shape[1] >= n_steps
    shared = _pack_weights(
        {k: np.asarray(v, np.float32) for k, v in inputs.items() if k != "x"},
        np_dt,
    )
    in_maps = []
    for c in range(ncores):
        xs = x[c * bl : (c + 1) * bl, :n_steps, :]  # [BL, T, 4]
        xT_np = np.ascontiguousarray(
            xs.transpose(1, 2, 0).reshape(n_steps * 4, bl)
        )
        in_maps.append(dict(shared, xT=xT_np.astype(np_dt)))
    nc = _get_nc(n_steps)
    res = bass_utils.run_bass_kernel_spmd(
        nc, in_maps, core_ids=list(range(ncores)), **run_kwargs
    )
    y = np.concatenate(
        [np.asarray(r["out"], np.float32).reshape(bl, 1) for r in res.results],
        axis=0,
    )
    return y, res


def kernel(**inputs) -> np.ndarray:
    y, _ = _run(inputs)
    return y


# revision 7
# speedup vs baseline: 1.1524x; 1.0407x over previous
"""Trainium2 Bass kernel for a 2-layer LSTM (64, 32) + MLP head.

Model (PyTorch semantics, eval mode):
    h1 = LSTM(4 -> 64)(x)            x: [B=4096, T=512, 4]
    h2 = LSTM(64 -> 32)(h1)
    y  = (relu(h2[:, -1] @ w_fc1.T + b_fc1)) @ w_fc2.T + b_fc2   # [B, 1]

Sharding: data-parallel over batch across 8 NeuronCores (512 rows each),
weights replicated.

v2 design (vs the v1 per-gate/PSUM-bank pipeline):
  * State kept transposed [units, batch]; layer-1 (64) and layer-2 (32)
    unit stacks fused to 96 rows with the 1-step layer-2 lag.
  * x folded INTO the recurrent matmul: the moving operand S is
    [101, F]: rows 0:96 h, row 96 ones (bias), rows 97:101 x_t.  One
    matmul per gate per step (K=101) instead of recurrent + x-proj
    pairs -- halves TensorE moving columns.
  * All 4 gates' matmuls write one contiguous PSUM tile [96, 4F], so
    ONE sigmoid activation instruction covers all four gates (ACT's
    ~185ns per-instruction access latency amortized 4x).  tanh(g) is
    computed as 2*sigmoid(2g)-1: the x2 is folded into g's weights, and
    the affine fix-up is a 4x-rate DVE tensor_scalar op.
  * Batch split into 2 independent chains of 256 (separate S/C/PSUM),
    interleaved in emission order so each engine works on one chain
    while the other chain's serial chain (mm -> sigmoid -> cell -> tanh
    -> h -> mm) is in flight on other engines.
  * S rotates over 4 buffers per chain; x_t DMA lands 4 steps ahead.
"""

import numpy as np
from contextlib import ExitStack

import concourse.bass as bass
import concourse.tile as tile
from concourse import bacc, mybir
from concourse import bass_utils
from concourse.alu_op_type import AluOpType

AF = mybir.ActivationFunctionType

B, T, D_IN, H1, H2 = 4096, 512, 4, 64, 32
NCORES = 8
BL = B // NCORES  # 512 batch rows per core

F32 = mybir.dt.float32
DT = mybir.dt.bfloat16

HS = H1 + H2          # 96 stacked units
KR = HS + 1 + D_IN    # 101 = h rows + ones row + x rows
NCH = 2               # batch chains per core
FC = BL // NCH        # 256 free columns per chain
NSB = 4               # S rotation depth
NDUMMY = 7            # PE-warming filler matmuls per step


def _build(n_steps: int = T):
    nc = bacc.Bacc("TRN2", target_bir_lowering=False, debug=False)

    xT = nc.dram_tensor("xT", [n_steps * 4, BL], DT, kind="ExternalInput")
    wg = nc.dram_tensor("wg", [KR, 4 * HS], DT, kind="ExternalInput")
    wf1 = nc.dram_tensor("wf1", [HS + 1, 16], DT, kind="ExternalInput")
    wf2 = nc.dram_tensor("wf2", [16, 1], DT, kind="ExternalInput")
    bf2 = nc.dram_tensor("bf2", [1, 1], F32, kind="ExternalInput")
    out = nc.dram_tensor("out", [1, BL], F32, kind="ExternalOutput")

    with tile.TileContext(nc) as tc, ExitStack() as ctx:
        const = ctx.enter_context(tc.tile_pool(name="const", bufs=1))
        sg_pool = ctx.enter_context(tc.tile_pool(name="sg", bufs=2))
        cell = ctx.enter_context(tc.tile_pool(name="cell", bufs=2))

        W = const.tile([KR, 4 * HS], DT, tag="W")
        nc.sync.dma_start(W[:], wg.ap())
        WF1 = const.tile([HS + 1, 16], DT, tag="WF1")
        nc.sync.dma_start(WF1[:], wf1.ap())
        WF2 = const.tile([16, 1], DT, tag="WF2")
        nc.sync.dma_start(WF2[:], wf2.ap())
        BF2 = const.tile([1, 1], F32, tag="BF2")
        nc.sync.dma_start(BF2[:], bf2.ap())

        # Per-chain state: S rotation ring and cell state C.
        S = [[None] * NSB for _ in range(NCH)]
        C = [None] * NCH
        for ch in range(NCH):
            for j in range(NSB):
                Sj = const.tile([KR, FC], DT, tag=f"S{ch}_{j}")
                nc.vector.memset(Sj[:], 0.0)
                nc.vector.memset(Sj[HS : HS + 1, :], 1.0)
                S[ch][j] = Sj
            Cch = const.tile([HS, FC], DT, tag=f"C{ch}")
            nc.vector.memset(Cch[:], 0.0)
            C[ch] = Cch

        # Preload x for steps 0..NSB-1 into the rings.
        for j in range(min(NSB, n_steps)):
            for ch in range(NCH):
                nc.sync.dma_start(
                    S[ch][j][HS + 1 : KR, :],
                    xT.ap()[4 * j : 4 * j + 4, ch * FC : (ch + 1) * FC],
                )

        # Gate order in W columns: i, f, g, o (each HS=96 wide).
        GI, GF, GG, GO = 0, 1, 2, 3

        with tc.tile_pool(name="psum", bufs=1, space="PSUM") as psum:
            P = [psum.tile([HS, 4 * FC], F32, tag=f"P{ch}", name=f"P{ch}") for ch in range(NCH)]
            # Scratch bank for PE-warming filler matmuls: the tensor engine
            # drops to its mid p-state (1.2 GHz) unless it stays busy ~3us;
            # filler matmuls during the per-step gate-recurrence gap keep it
            # at 2.4 GHz, halving the real matmuls on the critical loop.
            PDUM = psum.tile([HS, 4 * HS], F32, tag="PDUM", name="PDUM")

            for k in range(n_steps + 1):
                cur = [S[ch][k % NSB] for ch in range(NCH)]
                nxt = [S[ch][(k + 1) % NSB] for ch in range(NCH)]

                # 1) Recurrent+input matmuls: all 4 gates into one PSUM tile.
                for ch in range(NCH):
                    for g in range(4):
                        nc.tensor.matmul(
                            P[ch][:, g * FC : (g + 1) * FC],
                            W[:, g * HS : (g + 1) * HS],
                            cur[ch][0:KR, :],
                            start=True,
                            stop=True,
                        )
                # PE-warming fillers (see PDUM above); they execute during
                # the recurrence gap while PE waits for h(t).
                for _ in range(NDUMMY):
                    nc.tensor.matmul(
                        PDUM[:, :], W[:, 0:HS], W[0:KR, 0 : 4 * HS],
                        start=True, stop=True,
                    )
                # Prefetch x for step k+NSB into the buffer just freed.
                if k + NSB < n_steps:
                    for ch in range(NCH):
                        nc.sync.dma_start(
                            cur[ch][HS + 1 : KR, :],
                            xT.ap()[
                                4 * (k + NSB) : 4 * (k + NSB) + 4,
                                ch * FC : (ch + 1) * FC,
                            ],
                        )

                # 2) One sigmoid over all four gates per chain.
                SG = [None] * NCH
                for ch in range(NCH):
                    SGc = sg_pool.tile([HS, 4 * FC], DT, tag=f"SG{ch}")
                    nc.scalar.activation(SGc[:], P[ch][:, :], AF.Sigmoid)
                    SG[ch] = SGc

                # 3) Cell update on DVE (per chain, interleaved emission).
                GN = [None] * NCH
                UU = [None] * NCH
                VV = [None] * NCH
                for ch in range(NCH):
                    GNc = cell.tile([HS, FC], DT, tag=f"GN{ch}")
                    # g = tanh(g_pre) = 2*sigmoid(2*g_pre) - 1 (x2 in weights)
                    nc.vector.tensor_scalar(
                        GNc[:], SG[ch][:, GG * FC : (GG + 1) * FC],
                        2.0, 1.0, AluOpType.mult, AluOpType.subtract,
                    )
                    GN[ch] = GNc
                for ch in range(NCH):
                    Uc = cell.tile([HS, FC], DT, tag=f"U{ch}")
                    nc.vector.tensor_mul(
                        Uc[:], SG[ch][:, GI * FC : (GI + 1) * FC], GN[ch][:]
                    )
                    UU[ch] = Uc
                    Vc = cell.tile([HS, FC], DT, tag=f"V{ch}")
                    nc.vector.tensor_mul(
                        Vc[:], SG[ch][:, GF * FC : (GF + 1) * FC], C[ch][:]
                    )
                    VV[ch] = Vc
                    nc.vector.tensor_add(C[ch][:], UU[ch][:], VV[ch][:])

                # 4) tanh(c) on ACT, then h on DVE into the next S buffer.
                TC = [None] * NCH
                for ch in range(NCH):
                    TCc = cell.tile([HS, FC], DT, tag=f"TC{ch}")
                    nc.scalar.activation(TCc[:], C[ch][:], AF.Tanh)
                    TC[ch] = TCc
                for ch in range(NCH):
                    nc.vector.tensor_mul(
                        nxt[ch][0:HS, :], SG[ch][:, GO * FC : (GO + 1) * FC],
                        TC[ch][:],
                    )
                if k == 0:
                    # Wipe garbage layer-2 state from the pipeline warmup.
                    for ch in range(NCH):
                        nc.vector.memset(nxt[ch][H1:HS, :], 0.0)
                        nc.vector.memset(C[ch][H1:HS, :], 0.0)

        # MLP head on h2 of the final state (rows 64:96; wf1 zero over h1).
        final = [S[ch][(n_steps + 1) % NSB] for ch in range(NCH)]
        with tc.tile_pool(name="psum_head", bufs=1, space="PSUM") as psh:
            for ch in range(NCH):
                PF = psh.tile([16, FC], F32, tag=f"PF{ch}")
                nc.tensor.matmul(
                    PF[:], WF1[:, :], final[ch][0 : HS + 1, :],
                    start=True, stop=True,
                )
                Z = cell.tile([16, FC], DT, tag=f"Z{ch}")
                nc.scalar.activation(Z[:], PF[:], AF.Relu)
                PO = psh.tile([1, FC], F32, tag=f"PO{ch}")
                nc.tensor.matmul(PO[:], WF2[:, :], Z[:], start=True, stop=True)
                Y = cell.tile([1, FC], F32, tag=f"Y{ch}")
                nc.scalar.activation(Y[:], PO[:], AF.Identity, bias=BF2[:, 0:1])
                nc.sync.dma_start(out.ap()[:, ch * FC : (ch + 1) * FC], Y[:])

    nc.compile()
    return nc


def _pack_weights(inputs, np_dt):
    w_ih1, w_hh1 = inputs["w_ih1"], inputs["w_hh1"]
    w_ih2, w_hh2 = inputs["w_ih2"], inputs["w_hh2"]
    b1 = (inputs["b_ih1"] + inputs["b_hh1"]).astype(np.float32)
    b2 = (inputs["b_ih2"] + inputs["b_hh2"]).astype(np.float32)
    # Fused gate weight blocks [KR=101, 4*HS].  Rows: h1(64), h2(32),
    # ones/bias(1), x(4).  Columns gate-major i,f,g,o; within a gate,
    # layer-1 units then layer-2 units.  Layer 1 uses h1+x rows; layer 2
    # uses h1 (=w_ih2) + h2 rows and no x.  g gate scaled by 2 so that
    # tanh(g) = 2*sigmoid(2g) - 1 needs only sigmoid.
    blocks = []
    for g in range(4):
        w1 = np.concatenate(
            [
                w_hh1[g * H1 : (g + 1) * H1, :].T,           # h1 rows [64,64]
                np.zeros((H2, H1), np.float32),              # h2 rows
                b1[None, g * H1 : (g + 1) * H1],             # bias row
                w_ih1[g * H1 : (g + 1) * H1, :].T,           # x rows [4,64]
            ],
            axis=0,
        )
        w2 = np.concatenate(
            [
                w_ih2[g * H2 : (g + 1) * H2, :].T,           # h1 rows [64,32]
                w_hh2[g * H2 : (g + 1) * H2, :].T,           # h2 rows [32,32]
                b2[None, g * H2 : (g + 1) * H2],             # bias row
                np.zeros((D_IN, H2), np.float32),            # x rows
            ],
            axis=0,
        )
        blk = np.concatenate([w1, w2], axis=1)               # [101, 96]
        if g == 2:
            blk = blk * 2.0
        blocks.append(blk)
    wg = np.concatenate(blocks, axis=1)                      # [101, 384]
    return {
        "wg": np.ascontiguousarray(wg).astype(np_dt),
        "wf1": np.ascontiguousarray(
            np.concatenate(
                [np.zeros((H1, 16), np.float32), inputs["w_fc1"].T,
                 inputs["b_fc1"][None, :]], axis=0)
        ).astype(np_dt),
        "wf2": np.ascontiguousarray(inputs["w_fc2"].T).astype(np_dt),
        "bf2": np.ascontiguousarray(inputs["b_fc2"][:, None]).astype(np.float32),
    }


_built = {}


def _get_nc(n_steps):
    if n_steps not in _built:
        _built[n_steps] = _build(n_steps)
    return _built[n_steps]


def _run(inputs, n_steps=T, **run_kwargs):
    np_dt = mybir.dt.np(DT)
    x = np.asarray(inputs["x"], np.float32)
    nb = x.shape[0]
    ncores = NCORES
    bl = nb // ncores
    assert bl == BL and x.shape[1] >= n_steps
    shared = _pack_weights(
        {k: np.asarray(v, np.float32) for k, v in inputs.items() if k != "x"},
        np_dt,
    )
    in_maps = []
    for c in range(ncores):
        xs = x[c * bl : (c + 1) * bl, :n_steps, :]  # [BL, T, 4]
        xT_np = np.ascontiguousarray(
            xs.transpose(1, 2, 0).reshape(n_steps * 4, bl)
        )
        in_maps.append(dict(shared, xT=xT_np.astype(np_dt)))
    nc = _get_nc(n_steps)
    res = bass_utils.run_bass_kernel_spmd(
        nc, in_maps, core_ids=list(range(ncores)), **run_kwargs
    )
    y = np.concatenate(
        [np.asarray(r["out"], np.float32).reshape(bl, 1) for r in res.results],
        axis=0,
    )
    return y, res


def kernel(**inputs) -> np.ndarray:
    y, _ = _run(inputs)
    return y
